# revision 4
# baseline (speedup 1.0000x reference)
"""Bass/Tile kernel for nn_AttentionLayer (B,T,M,D)=(8,256,64,512), H=8, DK=64.

Data-parallel over batch: core c processes x[c] as a (16384, 512) row shard in
groups of 512 rows (= 8 pairs of M=64). All matmuls bf16 (fp32 PSUM).

Deep-pipeline redesign (HW round-trip latency between dependent cross-engine
ops is huge unless hidden ~8 deep):
 - ONE 8-slot PSUM pool; every PSUM tile is 1 bank; consumers trail ~8 banks.
 - Pair-pure score tiles (128 k-rows x [8 heads x 64 q]): no cross-pair
   garbage, no memsets, exp = ONE contiguous full-partition ACT op per u-step.
 - 4-stage group skew: proj(g) | scores+exp(g-1) | attn.V+norm(g-2) |
   o-proj+store(g-3); x is DMA-prefetched one group ahead.
 - bv is folded through attention into bo' = bo + Wo@bv; bias-free v.
"""

import sys

for _p in ("/opt/trn_rl_repo", "/opt/pypackages"):
    if _p not in sys.path:
        sys.path.append(_p)

from contextlib import ExitStack

import numpy as np

import concourse.bass as bass
import concourse.tile as tile
from concourse import bacc, mybir
from concourse import bass_utils
from concourse.bass import ts

F32 = mybir.dt.float32
BF16 = mybir.dt.bfloat16

B, T, M, D = 8, 256, 64, 512
H, DK = 8, 64
N_CORES = 8
ROWS = B * T * M // N_CORES  # 16384 rows per core
GROUP = 512                  # rows per group = 8 pairs of 64


def build_body(ctx: ExitStack, tc: tile.TileContext, io: dict, n_groups: int, repeat: int = 1):
    nc = tc.nc
    x = io["x"]          # (rows, 512) f32
    out = io["out"]      # (rows, 512) f32

    consts = ctx.enter_context(tc.tile_pool(name="consts", bufs=1))
    setup_sb = ctx.enter_context(tc.tile_pool(name="setup", bufs=4))

    # ONE PSUM pool: every tile is one bank, 8 banks rotate.
    psum = ctx.enter_context(
        tc.tile_pool(name="psum", bufs=8, space=bass.MemorySpace.PSUM))

    p_xin = ctx.enter_context(tc.tile_pool(name="xin", bufs=3))
    p_xbf = ctx.enter_context(tc.tile_pool(name="xbf", bufs=3))
    p_xTall = ctx.enter_context(tc.tile_pool(name="xTall", bufs=3))
    p_qT = ctx.enter_context(tc.tile_pool(name="qT", bufs=10))
    p_kT = ctx.enter_context(tc.tile_pool(name="kT", bufs=10))
    p_v = ctx.enter_context(tc.tile_pool(name="v", bufs=14))
    p_E = ctx.enter_context(tc.tile_pool(name="E", bufs=14))
    p_R = ctx.enter_context(tc.tile_pool(name="R", bufs=10))
    p_ao = ctx.enter_context(tc.tile_pool(name="ao", bufs=3))
    p_aoT = ctx.enter_context(tc.tile_pool(name="aoT", bufs=3))
    p_y = ctx.enter_context(tc.tile_pool(name="y", bufs=3))

    # ---------------- setup: weights (as W^T bf16), biases
    WTc = {}
    WT = {}
    for wname in ("Wq", "Wk", "Wv", "Wo"):
        w_ap = io[wname]  # (512, 512) f32, row-major (d_out, d_in)
        WTc[wname] = [[None] * 4 for _ in range(4)]
        for i in range(4):  # d_out block i
            wf = setup_sb.tile([128, 512], F32, tag="wload", name="wload")
            nc.sync.dma_start(wf[:], w_ap[ts(i, 128), :])
            wb = setup_sb.tile([128, 512], BF16, tag="wcast", name="wcast")
            nc.vector.tensor_copy(wb[:], wf[:])
            for c in range(4):  # d_in block c
                wt = consts.tile([128, 128], BF16, tag=f"WTc_{wname}_{c}_{i}",
                                 name=f"WTc_{wname}_{c}_{i}")
                nc.sync.dma_start_transpose(wt[:], wb[:, ts(c, 128)])
                WTc[wname][c][i] = wt
        if wname in ("Wv", "Wo"):
            WT[wname] = []
            for c in range(4):
                w512 = consts.tile([128, 512], BF16, tag=f"WT_{wname}_{c}",
                                   name=f"WT_{wname}_{c}")
                for i in range(4):
                    nc.vector.tensor_copy(w512[:, ts(i, 128)], WTc[wname][c][i][:])
                WT[wname].append(w512)

    def load_bias_cols(name):
        t_sb = consts.tile([128, 4], F32, tag=f"bT_{name}", name=f"bT_{name}")
        nc.sync.dma_start(t_sb[:], io[name].rearrange("(t p) -> p t", p=128))
        return t_sb

    bqT = load_bias_cols("bq")
    bqTs = consts.tile([128, 4], F32)
    nc.vector.tensor_scalar_mul(bqTs[:], bqT[:], 1.0 / np.sqrt(DK))
    bkT = load_bias_cols("bk")
    bvT = load_bias_cols("bv")
    bvT_bf = consts.tile([128, 4], BF16)
    nc.vector.tensor_copy(bvT_bf[:], bvT[:])

    # bo' = bo + Wo @ bv, broadcast across partitions via a DRAM roundtrip.
    bo_sb = consts.tile([1, 512], F32)
    nc.sync.dma_start(bo_sb[:], io["bo"].rearrange("(o d) -> o d", o=1))
    ps_b = psum.tile([1, 512], F32, tag="ps", padded_shape=[128, 512])
    for c in range(4):
        nc.tensor.matmul(ps_b[:], bvT_bf[:, c:c + 1], WT["Wo"][c][:],
                         start=(c == 0), stop=(c == 3))
    bo1 = consts.tile([1, 512], F32)
    nc.vector.tensor_add(bo1[:], ps_b[:], bo_sb[:])
    bo_dram = nc.dram_tensor("bo_scratch", [1, 512], F32).ap()
    nc.sync.dma_start(bo_dram[:, :], bo1[:])
    bo_bcast = consts.tile([128, 512], F32)
    nc.sync.dma_start(
        bo_bcast[:],
        bass.AP(tensor=bo_dram.tensor, offset=0, ap=[[0, 128], [1, 512]]))

    # ---------------- per-group stage emitters (4-stage skew)
    state = {}

    def x_chain(gi):
        g = gi % n_groups
        r0 = g * GROUP
        xin3 = p_xin.tile([128, 4, 512], F32, tag="xin", name="xin")
        nc.sync.dma_start(
            xin3[:], x[r0: r0 + GROUP, :].rearrange("(rb p) d -> p rb d", p=128))
        xbf3 = p_xbf.tile([128, 4, 512], BF16, tag="xbf", name="xbf")
        nc.gpsimd.tensor_copy(xbf3[:], xin3[:])
        xTall = p_xTall.tile([128, 16, 128], BF16, tag="xTall", name="xTall")
        nc.sync.dma_start_transpose(xTall[:], xbf3.rearrange("p r d -> p (r d)"))
        st = {"g": g, "xTr": xTall.rearrange("p (r c) i -> p r c i", c=4),
              "qT": [None] * 4, "kT": [None] * 4, "v": [None] * 4,
              "E": [None] * 4}
        state[gi] = st
        return st

    def q_quad(st, t):
        xTr = st["xTr"]
        ps = psum.tile([128, 512], F32, tag="ps", name="mmq")
        for c in range(4):
            nc.tensor.matmul(ps[:], WTc["Wq"][c][t][:], xTr[:, :, c, :],
                             start=(c == 0), stop=(c == 3))
        qt = p_qT.tile([128, 512], BF16, tag="qT", name="qT")
        nc.scalar.activation(qt[:], ps[:],
                             mybir.ActivationFunctionType.Identity,
                             bias=bqTs[:, t:t + 1], scale=1.0 / np.sqrt(DK))
        st["qT"][t] = qt

    def k_quad(st, t):
        xTr = st["xTr"]
        ps = psum.tile([128, 512], F32, tag="ps", name="mmk")
        for c in range(4):
            nc.tensor.matmul(ps[:], WTc["Wk"][c][t][:], xTr[:, :, c, :],
                             start=(c == 0), stop=(c == 3))
        kt = p_kT.tile([128, 512], BF16, tag="kT", name="kT")
        nc.scalar.activation(kt[:], ps[:],
                             mybir.ActivationFunctionType.Identity,
                             bias=bkT[:, t:t + 1], scale=1.0)
        st["kT"][t] = kt

    def v_quad(st, u):
        xTr = st["xTr"]
        ps = psum.tile([128, 512], F32, tag="ps", name="mmv")
        for c in range(4):
            nc.tensor.matmul(ps[:], xTr[:, u, c, :], WT["Wv"][c][:],
                             start=(c == 0), stop=(c == 3))
        vt = p_v.tile([128, 8, 65], BF16, tag="v", name="v")
        nc.gpsimd.memset(vt[:, :, 64:65], 1.0)
        nc.vector.tensor_copy(
            vt[:, :, 0:64], ps[:].rearrange("p (h c) -> p h c", c=64))
        st["v"][u] = vt

    def sc_only(st, u):
        """Pair-pure scores for u-span (2 pairs): per dk-parity tile
        sc_p[k-row(128), head-duo t(4), q(64)]. Two tiles so that every
        matmul in a bank shares one contraction row-base (HW constraint:
        same out-partition range + different row-base in a bank faults)."""
        qT, kT = st["qT"], st["kT"]
        sc_tiles = []
        for s in range(2):           # pair within the u-span
            col0 = u * 128 + s * 64
            pr = slice(s * 64, (s + 1) * 64)
            for hp in range(2):      # dk parity (head parity)
                dk = slice(hp * 64, (hp + 1) * 64)
                sc = psum.tile([128, 4, 64], F32, tag="ps", name="sc")
                for t in range(4):
                    nc.tensor.matmul(sc[pr, t, :],
                                     kT[t][dk, col0:col0 + 64],
                                     qT[t][dk, col0:col0 + 64],
                                     start=True, stop=True)
                sc_tiles.append((sc, pr, hp))
        st["sc"] = sc_tiles

    def exp_only(st, u):
        """exp: sc (PSUM f32) -> E bf16 [k-row, h, q], h = 2t + parity."""
        sc_tiles = st.pop("sc")
        E = p_E.tile([128, 8, 64], BF16, tag="E", name="E")
        Ev = E.rearrange("p (t o) c -> p t o c", o=2)
        for sc, pr, hp in sc_tiles:
            nc.scalar.activation(Ev[pr, :, hp, :], sc[pr, :, :],
                                 mybir.ActivationFunctionType.Exp)
        st["E"][u] = E

    def av_norm(st, u):
        """attn @ v for u-span; heads 0-3 -> av0, 4-7 -> av1 (1 bank each,
        65th col = softmax denominator); normalize into ao."""
        E, vt = st["E"][u], st["v"][u]
        if st.get("ao") is None:
            st["ao"] = p_ao.tile([128, 4, 8, 64], BF16, tag="ao", name="ao")
        ao = st["ao"]
        # one tile_position per bank: 4 av tiles per u-step, each single-pair
        # single-head-quad; full-bank stride so base-64 slices stay legal.
        for s in range(2):
            pr = slice(s * 64, (s + 1) * 64)
            for half in range(2):
                av = psum.tile([128, 4, 128], F32, tag="ps", name="av")
                for hq in range(4):
                    h = 4 * half + hq
                    nc.tensor.matmul(av[pr, hq, 0:65],
                                     E[pr, h, :], vt[pr, h, :],
                                     start=True, stop=True)
                R = p_R.tile([128, 4, 1], F32, tag="R", name="R")
                nc.vector.reciprocal(R[pr, :, :], av[pr, :, 64:65])
                nc.vector.tensor_mul(
                    ao[pr, u, 4 * half:4 * half + 4, :], av[pr, :, 0:64],
                    bass.AP(tensor=R.tensor, offset=R[pr, 0, 0].offset,
                            ap=[*R[pr, :, 0].ap, [0, 64]]))

    def ao_transpose(st):
        aoTall = p_aoT.tile([128, 16, 128], BF16, tag="aoT", name="aoT")
        nc.scalar.dma_start_transpose(
            aoTall[:], st["ao"].rearrange("p u h c -> p (u h c)"))
        st["aoTr"] = aoTall.rearrange("p (u c) i -> p u c i", c=4)

    def o_quad(st, u):
        if st.get("y") is None:
            st["y"] = p_y.tile([128, 4, 512], F32, tag="y", name="y")
        aoTr = st["aoTr"]
        ps = psum.tile([128, 512], F32, tag="ps", name="mmo")
        for c in range(4):
            nc.tensor.matmul(ps[:], aoTr[:, u, c, :], WT["Wo"][c][:],
                             start=(c == 0), stop=(c == 3))
        nc.vector.tensor_add(st["y"][:, u, :], ps[:], bo_bcast[:])

    def store(st, gi):
        g = st["g"]
        r0 = g * GROUP
        nc.scalar.dma_start(
            out[r0: r0 + GROUP, :].rearrange("(ub p) d -> p ub d", p=128),
            st["y"][:])
        del state[gi]

    # ---------------- emission: 4-stage software pipeline
    n_total = n_groups * repeat
    x_chain(0)
    for step in range(n_total + 3):
        stP = state.get(step)          # projections
        stS = state.get(step - 1)      # scores + exp
        stA = state.get(step - 2)      # attn@v + normalize
        stO = state.get(step - 3)      # o-proj + store
        for u in range(4):
            if stS is not None:
                sc_only(stS, u)
            if stP is not None:
                q_quad(stP, u)
            if stA is not None:
                av_norm(stA, u)
            if stS is not None:
                exp_only(stS, u)
            if stP is not None:
                k_quad(stP, u)
            if stO is not None:
                o_quad(stO, u)
            if stP is not None:
                v_quad(stP, u)
        if stA is not None:
            ao_transpose(stA)
        if stO is not None:
            store(stO, step - 3)
        if step + 1 < n_total:
            x_chain(step + 1)


_BUILD_CACHE = {}


def build_module(n_groups=ROWS // GROUP, repeat=1):
    if (n_groups, repeat) in _BUILD_CACHE:
        return _BUILD_CACHE[(n_groups, repeat)]
    rows = n_groups * GROUP
    nc = bacc.Bacc("TRN2", target_bir_lowering=False, debug=False)
    io = {
        "x": nc.dram_tensor("x", [rows, D], F32, kind="ExternalInput").ap(),
        "out": nc.dram_tensor("out", [rows, D], F32, kind="ExternalOutput").ap(),
    }
    for wname in ("Wq", "Wk", "Wv", "Wo"):
        io[wname] = nc.dram_tensor(wname, [D, D], F32, kind="ExternalInput").ap()
    for bname in ("bq", "bk", "bv", "bo"):
        io[bname] = nc.dram_tensor(bname, [D], F32, kind="ExternalInput").ap()

    with tile.TileContext(nc) as tc:
        with ExitStack() as ctx:
            build_body(ctx, tc, io, n_groups, repeat)
    nc.compile()
    _BUILD_CACHE[(n_groups, repeat)] = (nc, io)
    return nc, io


def kernel(x, Wq, bq, Wk, bk, Wv, bv, Wo, bo):
    x = np.ascontiguousarray(np.asarray(x, dtype=np.float32))
    weights = {
        "Wq": np.ascontiguousarray(np.asarray(Wq, dtype=np.float32)),
        "Wk": np.ascontiguousarray(np.asarray(Wk, dtype=np.float32)),
        "Wv": np.ascontiguousarray(np.asarray(Wv, dtype=np.float32)),
        "Wo": np.ascontiguousarray(np.asarray(Wo, dtype=np.float32)),
        "bq": np.ascontiguousarray(np.asarray(bq, dtype=np.float32)),
        "bk": np.ascontiguousarray(np.asarray(bk, dtype=np.float32)),
        "bv": np.ascontiguousarray(np.asarray(bv, dtype=np.float32)),
        "bo": np.ascontiguousarray(np.asarray(bo, dtype=np.float32)),
    }
    nc, _ = build_module()
    x_sh = x.reshape(N_CORES, ROWS, D)
    in_maps = [{"x": x_sh[c], **weights} for c in range(N_CORES)]
    res = bass_utils.run_bass_kernel_spmd(nc, in_maps, core_ids=list(range(N_CORES)))
    outs = [res.results[c]["out"] for c in range(N_CORES)]
    return np.stack(outs).reshape(B, T, M, D).astype(np.float32)


if __name__ == "__main__":
    build_module(4)
    print("build ok")
